# revision 43
# baseline (speedup 1.0000x reference)
"""RNN-T joint network kernel for 8 Trainium2 NeuronCores.

logits[b,t,u,v] = tanh(enc_out[b,t]@W_enc + b_enc + pred_out[b,u]@W_pred + b_pred) @ W_joint + b_joint

Sharding: (B x T-half) split 8 ways -- core c computes b = c//2,
t in [(c%2)*128, (c%2)*128+128). Each core emits its (128 t, 64 u, 1024 v)
logit slab; stacking the 8 per-core slabs along rows IS the final
(4, 256, 64, 1024) tensor (gather is a reshape, no transpose).

Per-core dataflow (activations kept transposed, J on partitions, bf16
compute with f32 PSUM accumulate and f32 output):
  loads        : need-ordered small DMAs; latency-critical pieces (packA,
                 w_joint vh0) on the ACT HWDGE ring, bulk (packB, bias_j,
                 w_joint vh1 tail) on the gpsimd SWDGE queue; the SP HWDGE
                 ring is reserved for output stores
  encT/predT   : transposed on the HOST in make_in_maps, DMA'd directly
                 in [E-part, row] layout (no on-device transposes)
  enc/pred proj: PE bf16 matmuls; weights host-packed so the jc=0 chunks
                 land first and proj-jc0 starts ~1.5us in
  joint        : DVE broadcast-add (0-step APs) + ACT tanh -> bf16 slab;
                 slab0+slab1 ride per-jc behind the projections, later
                 slabs are produced two groups ahead of their tiles
  prologue     : vh0 of the first two groups runs jc-OUTER across 8 open
                 PSUM banks, so PE starts on (slab jc0, w_joint vh0-jc0)
                 after only ~640KB of loads and the w_joint vh1 arrival
                 hides behind ~7us of matmuls
  logits       : PE bf16 matmuls, w_joint streaming as moving operand;
                 PSUM-evac + b_joint (bf16) add on DVE for even tiles,
                 ACT-copy + GPSIMD-add for odd (all-DVE for the last 8)
  store        : full 512KB row-block HWDGE DMA once both vocab halves of
                 a tile are evac'd (half the issue/semaphore cost); the
                 last group's stores drain on the by-then-idle ACT ring
                 (splitting the final ~3.5MB drain across both rings) and
                 the final tile stores in small chunks, so the kernel-end
                 store+receipt chain is as short as possible

_build(loop_k=K) wraps the whole body in a hardware For_i loop executing
the identical kernel K times back-to-back -- used by test.py to measure
steady-state per-iteration HW time with the dispatch overhead cancelled.
"""

import sys

for _p in ("/opt/trn_rl_repo", "/root/.axon_site/_ro/trn_rl_repo"):
    if _p not in sys.path:
        sys.path.insert(0, _p)

import numpy as np

import concourse.tile as tile
from concourse import bacc, mybir
from concourse import bass_utils
from concourse.ap import AP

F32 = mybir.dt.float32
BF16 = mybir.dt.bfloat16
NP_BF16 = mybir.dt.np(BF16)
TANH = mybir.ActivationFunctionType.Tanh
ADD = mybir.AluOpType.add

N_CORES = 8
B, T, U = 4, 256, 64
TL = T // 2                # 128 t's per core (half of one b)
E = P = J = 512
V = 1024
EC = E // 128              # 4 e-chunks (contraction for projections)
JC = J // 128              # 4 j-chunks (contraction for joint matmul)
NROW = TL * U              # 8192 output rows per core
NT = NROW // 128           # 64 output tiles of 128 rows
TG = 16                    # t-groups (8 t's each -> slab of 512 rows)

_cache = {}


def _build(loop_k=None, no_store=False, const_slab=False, hoist_loads=False):
    nc = bacc.Bacc("TRN2", target_bir_lowering=False, debug=False,
                   num_devices=N_CORES)

    # packA: everything the jc=0 projection + first slab chunk needs:
    # [encT (EC*TL) | w_enc jc0 (EC*128) | predT (EC*U) | w_pred jc0
    # (EC*128) | bias_ep (JC, bf16)], loaded as two DMAs
    PA = EC * TL + EC * U + 2 * EC * 128 + JC
    packA_d = nc.dram_tensor("packA", [128, PA], BF16, kind="ExternalInput").ap()
    # packB: w_enc jc1-3 | w_pred jc1-3
    PB = 2 * (JC - 1) * EC * 128
    packB_d = nc.dram_tensor("packB", [128, PB], BF16, kind="ExternalInput").ap()
    w_joint_d = nc.dram_tensor("w_joint", [128, 2 * JC * 512], BF16,
                               kind="ExternalInput").ap()
    bias_j_d = nc.dram_tensor("bias_j", [128, V], BF16, kind="ExternalInput").ap()
    out_d = nc.dram_tensor("out", [NROW, V], F32, kind="ExternalOutput").ap()

    with tile.TileContext(nc) as tc:
        with (
            tc.tile_pool(name="const", bufs=1) as cp,
            tc.tile_pool(name="psum_mm", bufs=8, space="PSUM") as ps_mm,
            tc.tile_pool(name="slab", bufs=4) as slab_pool,
            tc.tile_pool(name="outp", bufs=12) as out_pool,
        ):
            import contextlib

            def emit_loads():
                # ---- constant loads: small, need-ordered DMAs (fixed
                # ~0.6us HWDGE issue cost each) so each pipeline stage's
                # bytes land just-in-time ----
                o0 = EC * TL            # end of encT
                o1 = o0 + EC * 128      # end of w_enc jc0
                o2 = o1 + EC * U        # end of predT
                o3 = o2 + EC * 128      # end of w_pred jc0
                # all loads on the ACT HWDGE ring so they never share a
                # FIFO with the output stores (which own the SP ring)
                packA_sb = cp.tile([128, PA], BF16)
                nc.scalar.dma_start(packA_sb[:, 0:o1], packA_d[:, 0:o1])
                nc.scalar.dma_start(packA_sb[:, o1:PA], packA_d[:, o1:PA])
                w_joint_sb = cp.tile([128, 2, JC, 512], BF16)
                wj_re = w_joint_d.rearrange("p (h c v) -> p h c v", h=2, c=JC)
                nc.scalar.dma_start(w_joint_sb[:, 0, 0], wj_re[:, 0, 0])
                packB_sb = cp.tile([128, PB], BF16)
                nc.gpsimd.dma_start(packB_sb[:, 0:PB // 2], packB_d[:, 0:PB // 2])
                nc.gpsimd.dma_start(packB_sb[:, PB // 2:PB], packB_d[:, PB // 2:PB])
                nc.scalar.dma_start(w_joint_sb[:, 0, 1:4], wj_re[:, 0, 1:4])
                bias_j_sb = cp.tile([128, V], BF16)
                nc.gpsimd.dma_start(bias_j_sb[:], bias_j_d[:])
                nc.scalar.dma_start(w_joint_sb[:, 1, 0], wj_re[:, 1, 0])
                nc.gpsimd.dma_start(w_joint_sb[:, 1, 1:4], wj_re[:, 1, 1:4])

                # views into the packs
                encT = packA_sb[:, 0:o0].rearrange("p (e t) -> p e t", e=EC)
                we0 = packA_sb[:, o0:o1].rearrange("p (e k) -> p e k", e=EC)
                predT = packA_sb[:, o1:o2].rearrange("p (e u) -> p e u", e=EC)
                wp0 = packA_sb[:, o2:o3].rearrange("p (e k) -> p e k", e=EC)
                bias_ep_sb = packA_sb[:, o3:o3 + JC]
                web = packB_sb[:, 0:PB // 2].rearrange(
                    "p (c e k) -> p c e k", c=JC - 1, e=EC)
                wpb = packB_sb[:, PB // 2:PB].rearrange(
                    "p (c e k) -> p c e k", c=JC - 1, e=EC)

                def w_enc_ap(jc, ec):
                    return we0[:, ec, :] if jc == 0 else web[:, jc - 1, ec, :]

                def w_pred_ap(jc, ec):
                    return wp0[:, ec, :] if jc == 0 else wpb[:, jc - 1, ec, :]

                return (encT, predT, w_enc_ap, w_pred_ap, bias_ep_sb,
                        w_joint_sb, bias_j_sb)

            def emit_compute(views):
                (encT, predT, w_enc_ap, w_pred_ap, bias_ep_sb,
                 w_joint_sb, bias_j_sb) = views
                # ---- projections (transposed layout [j, row]), interleaved
                # per-jc so the first slab chunk is ready ASAP ----
                encP = cp.tile([128, JC, TL], BF16)
                predP = cp.tile([128, JC, U], BF16)
                slab0 = slab_pool.tile([128, JC, 512], BF16)
                slab1 = slab_pool.tile([128, JC, 512], BF16)

                def slab_add_tanh(slab, tg, jc):
                    t0 = tg * 8
                    # [128, 8t, 64u] = pred[., u] (bcast t) + enc[., t] (bcast u)
                    p_ap = predP[:, jc, :]
                    in0 = AP(p_ap.tensor, p_ap.offset,
                             [p_ap.ap[0], [0, 8], [1, U]])
                    e_ap = encP[:, jc, t0:t0 + 8]
                    in1 = AP(e_ap.tensor, e_ap.offset,
                             [e_ap.ap[0], [1, 8], [0, U]])
                    dst = slab[:, jc, :].rearrange("p (t u) -> p t u", t=8)
                    nc.vector.tensor_tensor(dst, in0, in1, ADD)
                    nc.scalar.activation(slab[:, jc, :], slab[:, jc, :], TANH)

                for jc in range(JC):
                    pe = ps_mm.tile([128, 512], F32, tag="mm")
                    for ec in range(EC):
                        nc.tensor.matmul(pe[:, 0:TL],
                                         w_enc_ap(jc, ec),
                                         encT[:, ec, :],
                                         start=(ec == 0), stop=(ec == EC - 1))
                    nc.scalar.copy(encP[:, jc, :], pe[:, 0:TL])
                    pp = ps_mm.tile([128, 512], F32, tag="mm")
                    for ec in range(EC):
                        nc.tensor.matmul(pp[:, 0:U],
                                         w_pred_ap(jc, ec),
                                         predT[:, ec, :],
                                         start=(ec == 0), stop=(ec == EC - 1))
                    # fused (b_enc + b_pred) bias add during PSUM evacuation
                    # (bias broadcast along u via a 0-step AP)
                    b_ap = bias_ep_sb[:, jc:jc + 1]
                    b_bc = AP(b_ap.tensor, b_ap.offset, [b_ap.ap[0], [0, U]])
                    nc.vector.tensor_tensor(predP[:, jc, :], pp[:, 0:U],
                                            b_bc, ADD)
                    # first two slab chunks ride right behind their
                    # projections (both consumed by the jc-outer prologue)
                    slab_add_tanh(slab0, 0, jc)
                    slab_add_tanh(slab1, 1, jc)

                # ---- main loop: per t-group of 8 -> slab -> 4 out tiles.
                # Slabs are produced one group ahead of their tiles so
                # DVE/ACT slab work never gates the PE matmul stream. ----
                slabs = {0: slab0, 1: slab1}

                def make_slab(tg):
                    s = slab_pool.tile([128, JC, 512], BF16, name="slab",
                                       tag="slab")
                    for jc in range(JC):
                        slab_add_tanh(s, tg, jc)
                    slabs[tg] = s

                def evac_store(tg, pi, vh, po, ot):
                    tidx = tg * 4 + pi
                    vs = slice(vh * 512, (vh + 1) * 512)
                    if no_store:
                        # timing ablation: tiny evac+store keeps the full
                        # dependency chain alive but removes ~all DVE/DMA load
                        vs16 = slice(vh * 512, vh * 512 + 16)
                        nc.vector.tensor_tensor(
                            ot[:, vs16], po[:, 0:16], bias_j_sb[:, vs16], ADD)
                        nc.sync.dma_start(
                            out_d[tidx * 128:(tidx + 1) * 128, vs16],
                            ot[:, vs16])
                        return
                    if tidx == NT - 1 and vh == 1:
                        # final half-tile: evac+store in two 256-col chunks
                        # so the kernel-end store chain is as short as
                        # possible (smaller final transfer + receipt)
                        for h in range(2):
                            hs = slice(vh * 512 + h * 256,
                                       vh * 512 + (h + 1) * 256)
                            ps = slice(h * 256, (h + 1) * 256)
                            nc.vector.tensor_tensor(
                                ot[:, hs], po[:, ps], bias_j_sb[:, hs], ADD)
                            nc.scalar.dma_start(
                                out_d[tidx * 128:(tidx + 1) * 128, hs],
                                ot[:, hs])
                        return
                    if tidx % 2 == 0 or tidx >= NT - 8:
                        # DVE: fused PSUM evac + b_joint add
                        nc.vector.tensor_tensor(
                            ot[:, vs], po[:], bias_j_sb[:, vs], ADD)
                    else:
                        # ACT evacuates, GPSIMD adds bias in place
                        nc.scalar.copy(ot[:, vs], po[:])
                        nc.gpsimd.tensor_add(
                            ot[:, vs], ot[:, vs], bias_j_sb[:, vs])
                    if tidx == NT - 1:
                        # final tile: store vh0 as soon as it's ready, on
                        # the ACT ring (free of DMAs by now) so the
                        # kernel-end chain never queues behind the last
                        # 512KB stores draining on the SP ring
                        nc.scalar.dma_start(
                            out_d[tidx * 128:(tidx + 1) * 128, vs], ot[:, vs])
                    elif vh == 1:
                        # full 512KB row-block store once both halves are
                        # evac'd: half the HWDGE issues + sem receipts;
                        # last group drains on the ACT ring
                        eng = nc.scalar if tidx >= NT - 4 else nc.sync
                        eng.dma_start(
                            out_d[tidx * 128:(tidx + 1) * 128, :], ot[:, :])

                ots_all = {}

                def get_ot(tg, pi):
                    key = (tg, pi)
                    if key not in ots_all:
                        ots_all[key] = out_pool.tile([128, V], F32,
                                                     name="ot", tag="ot")
                    return ots_all[key]

                # ---- jc-outer prologue: vh0 of tg0+tg1 as 8 open PSUM
                # accumulation groups, so matmuls start on slab chunk jc as
                # soon as it (and the 128KB w_joint vh0-jc chunk) lands,
                # instead of waiting for full slabs + the full vh0 half.
                # ~7us of PE work off the first 640KB of loads; also hides
                # the w_joint vh1 DMA arrival. ----
                if not const_slab:
                    pairs = [(t_, pi) for t_ in (0, 1) for pi in range(4)]
                    pos = [ps_mm.tile([128, 512], F32, tag="mm",
                                      name=f"po{i}")
                           for i in range(8)]
                    for jc in range(JC):
                        for i, (t_, pi) in enumerate(pairs):
                            nc.tensor.matmul(
                                pos[i][:],
                                slabs[t_][:, jc, pi * 128:(pi + 1) * 128],
                                w_joint_sb[:, 0, jc, :],
                                start=(jc == 0), stop=(jc == JC - 1))
                    for i, (t_, pi) in enumerate(pairs):
                        evac_store(t_, pi, 0, pos[i], get_ot(t_, pi))

                for tg in range(TG):
                    if const_slab:
                        slab = slab0
                    else:
                        if tg + 2 < TG:
                            make_slab(tg + 2)
                        slab = slabs.pop(tg)

                    def half_tile(pi, vh):
                        ot = get_ot(tg, pi)
                        po = ps_mm.tile([128, 512], F32, tag="mm")
                        for jc in range(JC):
                            nc.tensor.matmul(
                                po[:],
                                slab[:, jc, pi * 128:(pi + 1) * 128],
                                w_joint_sb[:, vh, jc, :],
                                start=(jc == 0), stop=(jc == JC - 1))
                        evac_store(tg, pi, vh, po, ot)

                    if tg < 2 and not const_slab:
                        # vh0 already computed in the jc-outer prologue
                        order = [(pi, 1) for pi in range(4)]
                    else:
                        order = [(pi, vh) for pi in range(4) for vh in range(2)]
                    for pi, vh in order:
                        half_tile(pi, vh)

            loop_cm = tc.For_i(0, loop_k) if loop_k else contextlib.nullcontext()
            if hoist_loads and loop_k:
                views = emit_loads()
                with loop_cm:
                    emit_compute(views)
            else:
                with loop_cm:
                    emit_compute(emit_loads())
    nc.compile()
    return nc


def _get_nc():
    if "nc" not in _cache:
        _cache["nc"] = _build()
    return _cache["nc"]


def make_in_maps(enc_out, pred_out, W_enc, b_enc, W_pred, b_pred, W_joint, b_joint):
    enc_f32 = np.asarray(enc_out, dtype=np.float32)
    pred_f32 = np.asarray(pred_out, dtype=np.float32)
    # weights packed [p, jc, ec, 128]; jc=0 chunk goes in packA, rest in packB
    w_enc = np.asarray(W_enc, dtype=np.float32).reshape(EC, 128, JC, 128) \
        .transpose(1, 2, 0, 3).astype(NP_BF16)
    w_pred = np.asarray(W_pred, dtype=np.float32).reshape(EC, 128, JC, 128) \
        .transpose(1, 2, 0, 3).astype(NP_BF16)
    # w_joint packed [p, vh, jc, 512] so per-vh halves are contiguous DMAs
    w_joint = np.ascontiguousarray(
        np.asarray(W_joint, dtype=np.float32).reshape(JC, 128, 2, 512)
        .transpose(1, 2, 0, 3).reshape(128, 2 * JC * 512)).astype(NP_BF16)
    bias_ep = ((np.asarray(b_enc, dtype=np.float32)
                + np.asarray(b_pred, dtype=np.float32))
               .reshape(JC, 128).T).astype(NP_BF16)
    bias_j = np.ascontiguousarray(
        np.broadcast_to(np.asarray(b_joint, dtype=np.float32), (128, V))
    ).astype(NP_BF16)
    packB = np.ascontiguousarray(np.concatenate(
        [w_enc[:, 1:].reshape(128, -1), w_pred[:, 1:].reshape(128, -1)],
        axis=1))
    predT = {}
    for b in range(B):
        predT[b] = pred_f32[b].reshape(U, EC, 128).transpose(2, 1, 0) \
            .reshape(128, EC * U).astype(NP_BF16)
    in_maps = []
    packA_cache = {}
    for c in range(N_CORES):
        b, toff = c // 2, (c % 2) * TL
        key = (b, toff)
        if key not in packA_cache:
            enc_slab = enc_f32[b, toff:toff + TL, :].reshape(TL, EC, 128) \
                .transpose(2, 1, 0).reshape(128, EC * TL).astype(NP_BF16)
            packA_cache[key] = np.ascontiguousarray(np.concatenate(
                [enc_slab, w_enc[:, 0].reshape(128, -1), predT[b],
                 w_pred[:, 0].reshape(128, -1), bias_ep], axis=1))
        in_maps.append({
            "packA": packA_cache[key], "packB": packB,
            "w_joint": w_joint, "bias_j": bias_j,
        })
    return in_maps


def assemble(results):
    # core order is (b0,t0-127), (b0,t128-255), (b1,t0-127), ... so a
    # straight row-stack is already (B, T, U, V)
    return np.concatenate([r["out"] for r in results], axis=0).reshape(B, T, U, V)


def _axon_active():
    try:
        from concourse.bass_utils import axon_active
        return axon_active()
    except Exception:
        return False


def _get_fast_runner(nc):
    """Cached jit dispatch (axon path). Same mechanism as
    bass2jax.run_bass_via_pjrt, built once so repeat kernel() calls skip
    the per-call trace/lower/compile."""
    if "runner" in _cache:
        return _cache["runner"]

    import jax
    from jax.sharding import Mesh, PartitionSpec, NamedSharding
    from jax.experimental.shard_map import shard_map
    from concourse.bass2jax import (
        _bass_exec_p, install_neuronx_cc_hook, partition_id_tensor)

    install_neuronx_cc_hook()
    partition_name = nc.partition_id_tensor.name if nc.partition_id_tensor else None
    in_names, out_names, out_avals, zero_outs = [], [], [], []
    for alloc in nc.m.functions[0].allocations:
        if not isinstance(alloc, mybir.MemoryLocationSet):
            continue
        name = alloc.memorylocations[0].name
        if alloc.kind == "ExternalInput":
            if name != partition_name:
                in_names.append(name)
        elif alloc.kind == "ExternalOutput":
            shape = tuple(alloc.tensor_shape)
            dtype = mybir.dt.np(alloc.dtype)
            out_names.append(name)
            out_avals.append(jax.core.ShapedArray(shape, dtype))
            zero_outs.append(np.zeros(shape, dtype))
    n_params = len(in_names)
    n_outs = len(out_avals)
    all_names = in_names + out_names
    if partition_name is not None:
        all_names = all_names + [partition_name]

    def _body(*args):
        operands = list(args)
        if partition_name is not None:
            operands.append(partition_id_tensor())
        outs = _bass_exec_p.bind(
            *operands, out_avals=tuple(out_avals), in_names=tuple(all_names),
            out_names=tuple(out_names), lowering_input_output_aliases=(),
            sim_require_finite=True, sim_require_nnan=True, nc=nc)
        return tuple(outs)

    devices = jax.devices()[:N_CORES]
    mesh = Mesh(np.asarray(devices), ("core",))
    sharded = jax.jit(
        shard_map(_body, mesh=mesh,
                  in_specs=(PartitionSpec("core"),) * (n_params + n_outs),
                  out_specs=(PartitionSpec("core"),) * n_outs,
                  check_rep=False),
        keep_unused=True)
    sh = NamedSharding(mesh, PartitionSpec("core"))
    zeros_dev = [
        jax.device_put(np.zeros((N_CORES * z.shape[0], *z.shape[1:]), z.dtype), sh)
        for z in zero_outs]

    oi = out_names.index("out")

    def run(in_maps):
        concat_in = [
            jax.device_put(
                np.concatenate([in_maps[c][n] for c in range(N_CORES)], axis=0), sh)
            for n in in_names]
        outs = sharded(*concat_in, *zeros_dev)
        # core order is (b0,t0-127), (b0,t128-255), (b1,t0-127), ... so the
        # row-sharded global array is already (B*T*U, V): one host gather,
        # zero-copy reshape
        return np.asarray(outs[oi]).reshape(B, T, U, V)

    _cache["runner"] = run
    return run


def kernel(enc_out, pred_out, W_enc, b_enc, W_pred, b_pred, W_joint, b_joint):
    nc = _get_nc()
    in_maps = make_in_maps(enc_out, pred_out, W_enc, b_enc, W_pred, b_pred,
                           W_joint, b_joint)
    if _axon_active():
        return _get_fast_runner(nc)(in_maps)
    results = bass_utils.run_bass_kernel_spmd(
        nc, in_maps, list(range(N_CORES))).results
    return assemble(results)


# revision 47
# speedup vs baseline: 1.0039x; 1.0039x over previous
"""RNN-T joint network kernel for 8 Trainium2 NeuronCores.

logits[b,t,u,v] = tanh(enc_out[b,t]@W_enc + b_enc + pred_out[b,u]@W_pred + b_pred) @ W_joint + b_joint

Sharding: (B x T-half) split 8 ways -- core c computes b = c//2,
t in [(c%2)*128, (c%2)*128+128). Each core emits its (128 t, 64 u, 1024 v)
logit slab; stacking the 8 per-core slabs along rows IS the final
(4, 256, 64, 1024) tensor (gather is a reshape, no transpose).

Per-core dataflow (activations kept transposed, J on partitions, bf16
compute with f32 PSUM accumulate and f32 output):
  loads        : need-ordered small DMAs; latency-critical pieces (packA,
                 w_joint vh0) on the ACT HWDGE ring, bulk (packB, bias_j,
                 w_joint vh1 tail) on the gpsimd SWDGE queue; the SP HWDGE
                 ring is reserved for output stores
  encT/predT   : transposed on the HOST in make_in_maps, DMA'd directly
                 in [E-part, row] layout (no on-device transposes)
  enc/pred proj: PE bf16 matmuls; weights host-packed so the jc=0 chunks
                 land first and proj-jc0 starts ~1.5us in
  joint        : DVE broadcast-add (0-step APs) + ACT tanh -> bf16 slab;
                 slab0+slab1 ride per-jc behind the projections, later
                 slabs are produced two groups ahead of their tiles
  prologue     : tg0's vh0 runs jc-OUTER as 4 open PSUM groups with the
                 jc2/jc3 projections WOVEN BETWEEN rounds -- the PE queue
                 is in-order, so un-interleaved emission would park ready
                 prologue rounds behind proj matmuls stalled on the packB
                 arrival; PE starts after only ~640KB of loads
  logits       : PE bf16 matmuls, w_joint streaming as moving operand;
                 PSUM-evac + b_joint (bf16) add on DVE for even tiles,
                 ACT-copy + GPSIMD-add for odd (all-DVE for the last 8)
  store        : full 512KB row-block HWDGE DMA once both vocab halves of
                 a tile are evac'd (half the issue/semaphore cost); the
                 last group's stores drain on the by-then-idle ACT ring
                 (splitting the final ~3.5MB drain across both rings) and
                 the final tile stores in small chunks, so the kernel-end
                 store+receipt chain is as short as possible

_build(loop_k=K) wraps the whole body in a hardware For_i loop executing
the identical kernel K times back-to-back -- used by test.py to measure
steady-state per-iteration HW time with the dispatch overhead cancelled.
"""

import sys

for _p in ("/opt/trn_rl_repo", "/root/.axon_site/_ro/trn_rl_repo"):
    if _p not in sys.path:
        sys.path.insert(0, _p)

import numpy as np

import concourse.tile as tile
from concourse import bacc, mybir
from concourse import bass_utils
from concourse.ap import AP

F32 = mybir.dt.float32
BF16 = mybir.dt.bfloat16
NP_BF16 = mybir.dt.np(BF16)
TANH = mybir.ActivationFunctionType.Tanh
ADD = mybir.AluOpType.add

N_CORES = 8
B, T, U = 4, 256, 64
TL = T // 2                # 128 t's per core (half of one b)
E = P = J = 512
V = 1024
EC = E // 128              # 4 e-chunks (contraction for projections)
JC = J // 128              # 4 j-chunks (contraction for joint matmul)
NROW = TL * U              # 8192 output rows per core
NT = NROW // 128           # 64 output tiles of 128 rows
TG = 16                    # t-groups (8 t's each -> slab of 512 rows)

_cache = {}


def _build(loop_k=None, no_store=False, const_slab=False, hoist_loads=False):
    nc = bacc.Bacc("TRN2", target_bir_lowering=False, debug=False,
                   num_devices=N_CORES)

    # packA: everything the jc=0 projection + first slab chunk needs:
    # [encT (EC*TL) | w_enc jc0 (EC*128) | predT (EC*U) | w_pred jc0
    # (EC*128) | bias_ep (JC, bf16)], loaded as two DMAs
    PA = EC * TL + EC * U + 2 * EC * 128 + JC
    packA_d = nc.dram_tensor("packA", [128, PA], BF16, kind="ExternalInput").ap()
    # packB: w_enc jc1-3 | w_pred jc1-3
    PB = 2 * (JC - 1) * EC * 128
    packB_d = nc.dram_tensor("packB", [128, PB], BF16, kind="ExternalInput").ap()
    w_joint_d = nc.dram_tensor("w_joint", [128, 2 * JC * 512], BF16,
                               kind="ExternalInput").ap()
    bias_j_d = nc.dram_tensor("bias_j", [128, V], BF16, kind="ExternalInput").ap()
    out_d = nc.dram_tensor("out", [NROW, V], F32, kind="ExternalOutput").ap()

    with tile.TileContext(nc) as tc:
        with (
            tc.tile_pool(name="const", bufs=1) as cp,
            tc.tile_pool(name="psum_mm", bufs=8, space="PSUM") as ps_mm,
            tc.tile_pool(name="slab", bufs=4) as slab_pool,
            tc.tile_pool(name="outp", bufs=12) as out_pool,
        ):
            import contextlib

            def emit_loads():
                # ---- constant loads: small, need-ordered DMAs (fixed
                # ~0.6us HWDGE issue cost each) so each pipeline stage's
                # bytes land just-in-time ----
                o0 = EC * TL            # end of encT
                o1 = o0 + EC * 128      # end of w_enc jc0
                o2 = o1 + EC * U        # end of predT
                o3 = o2 + EC * 128      # end of w_pred jc0
                # all loads on the ACT HWDGE ring so they never share a
                # FIFO with the output stores (which own the SP ring)
                packA_sb = cp.tile([128, PA], BF16)
                nc.scalar.dma_start(packA_sb[:, 0:o1], packA_d[:, 0:o1])
                nc.scalar.dma_start(packA_sb[:, o1:PA], packA_d[:, o1:PA])
                w_joint_sb = cp.tile([128, 2, JC, 512], BF16)
                wj_re = w_joint_d.rearrange("p (h c v) -> p h c v", h=2, c=JC)
                nc.scalar.dma_start(w_joint_sb[:, 0, 0], wj_re[:, 0, 0])
                packB_sb = cp.tile([128, PB], BF16)
                nc.gpsimd.dma_start(packB_sb[:, 0:PB // 2], packB_d[:, 0:PB // 2])
                nc.gpsimd.dma_start(packB_sb[:, PB // 2:PB], packB_d[:, PB // 2:PB])
                nc.scalar.dma_start(w_joint_sb[:, 0, 1:4], wj_re[:, 0, 1:4])
                bias_j_sb = cp.tile([128, V], BF16)
                nc.gpsimd.dma_start(bias_j_sb[:], bias_j_d[:])
                nc.scalar.dma_start(w_joint_sb[:, 1, 0], wj_re[:, 1, 0])
                nc.gpsimd.dma_start(w_joint_sb[:, 1, 1:4], wj_re[:, 1, 1:4])

                # views into the packs
                encT = packA_sb[:, 0:o0].rearrange("p (e t) -> p e t", e=EC)
                we0 = packA_sb[:, o0:o1].rearrange("p (e k) -> p e k", e=EC)
                predT = packA_sb[:, o1:o2].rearrange("p (e u) -> p e u", e=EC)
                wp0 = packA_sb[:, o2:o3].rearrange("p (e k) -> p e k", e=EC)
                bias_ep_sb = packA_sb[:, o3:o3 + JC]
                web = packB_sb[:, 0:PB // 2].rearrange(
                    "p (c e k) -> p c e k", c=JC - 1, e=EC)
                wpb = packB_sb[:, PB // 2:PB].rearrange(
                    "p (c e k) -> p c e k", c=JC - 1, e=EC)

                def w_enc_ap(jc, ec):
                    return we0[:, ec, :] if jc == 0 else web[:, jc - 1, ec, :]

                def w_pred_ap(jc, ec):
                    return wp0[:, ec, :] if jc == 0 else wpb[:, jc - 1, ec, :]

                return (encT, predT, w_enc_ap, w_pred_ap, bias_ep_sb,
                        w_joint_sb, bias_j_sb)

            def emit_compute(views):
                (encT, predT, w_enc_ap, w_pred_ap, bias_ep_sb,
                 w_joint_sb, bias_j_sb) = views
                # ---- projections (transposed layout [j, row]), interleaved
                # per-jc so the first slab chunk is ready ASAP ----
                encP = cp.tile([128, JC, TL], BF16)
                predP = cp.tile([128, JC, U], BF16)
                slab0 = slab_pool.tile([128, JC, 512], BF16)
                slab1 = slab_pool.tile([128, JC, 512], BF16)

                def slab_add_tanh(slab, tg, jc):
                    t0 = tg * 8
                    # [128, 8t, 64u] = pred[., u] (bcast t) + enc[., t] (bcast u)
                    p_ap = predP[:, jc, :]
                    in0 = AP(p_ap.tensor, p_ap.offset,
                             [p_ap.ap[0], [0, 8], [1, U]])
                    e_ap = encP[:, jc, t0:t0 + 8]
                    in1 = AP(e_ap.tensor, e_ap.offset,
                             [e_ap.ap[0], [1, 8], [0, U]])
                    dst = slab[:, jc, :].rearrange("p (t u) -> p t u", t=8)
                    nc.vector.tensor_tensor(dst, in0, in1, ADD)
                    nc.scalar.activation(slab[:, jc, :], slab[:, jc, :], TANH)

                def proj_jc(jc):
                    pe = ps_mm.tile([128, 512], F32, tag="mm", name="pe")
                    for ec in range(EC):
                        nc.tensor.matmul(pe[:, 0:TL],
                                         w_enc_ap(jc, ec),
                                         encT[:, ec, :],
                                         start=(ec == 0), stop=(ec == EC - 1))
                    nc.scalar.copy(encP[:, jc, :], pe[:, 0:TL])
                    pp = ps_mm.tile([128, 512], F32, tag="mm", name="pp")
                    for ec in range(EC):
                        nc.tensor.matmul(pp[:, 0:U],
                                         w_pred_ap(jc, ec),
                                         predT[:, ec, :],
                                         start=(ec == 0), stop=(ec == EC - 1))
                    # fused (b_enc + b_pred) bias add during PSUM evacuation
                    # (bias broadcast along u via a 0-step AP)
                    b_ap = bias_ep_sb[:, jc:jc + 1]
                    b_bc = AP(b_ap.tensor, b_ap.offset, [b_ap.ap[0], [0, U]])
                    nc.vector.tensor_tensor(predP[:, jc, :], pp[:, 0:U],
                                            b_bc, ADD)
                    # first two slab chunks ride right behind their
                    # projections (both consumed by the jc-outer prologue)
                    slab_add_tanh(slab0, 0, jc)
                    slab_add_tanh(slab1, 1, jc)

                # ---- main loop: per t-group of 8 -> slab -> 4 out tiles.
                # Slabs are produced one group ahead of their tiles so
                # DVE/ACT slab work never gates the PE matmul stream. ----
                slabs = {0: slab0, 1: slab1}

                def make_slab(tg):
                    s = slab_pool.tile([128, JC, 512], BF16, name="slab",
                                       tag="slab")
                    for jc in range(JC):
                        slab_add_tanh(s, tg, jc)
                    slabs[tg] = s

                def evac_store(tg, pi, vh, po, ot):
                    tidx = tg * 4 + pi
                    vs = slice(vh * 512, (vh + 1) * 512)
                    if no_store:
                        # timing ablation: tiny evac+store keeps the full
                        # dependency chain alive but removes ~all DVE/DMA load
                        vs16 = slice(vh * 512, vh * 512 + 16)
                        nc.vector.tensor_tensor(
                            ot[:, vs16], po[:, 0:16], bias_j_sb[:, vs16], ADD)
                        nc.sync.dma_start(
                            out_d[tidx * 128:(tidx + 1) * 128, vs16],
                            ot[:, vs16])
                        return
                    if tidx == NT - 1 and vh == 1:
                        # final half-tile: evac+store in two 256-col chunks
                        # so the kernel-end store chain is as short as
                        # possible (smaller final transfer + receipt)
                        for h in range(2):
                            hs = slice(vh * 512 + h * 256,
                                       vh * 512 + (h + 1) * 256)
                            ps = slice(h * 256, (h + 1) * 256)
                            nc.vector.tensor_tensor(
                                ot[:, hs], po[:, ps], bias_j_sb[:, hs], ADD)
                            nc.scalar.dma_start(
                                out_d[tidx * 128:(tidx + 1) * 128, hs],
                                ot[:, hs])
                        return
                    if tidx % 2 == 0 or tidx >= NT - 8:
                        # DVE: fused PSUM evac + b_joint add
                        nc.vector.tensor_tensor(
                            ot[:, vs], po[:], bias_j_sb[:, vs], ADD)
                    else:
                        # ACT evacuates, GPSIMD adds bias in place
                        nc.scalar.copy(ot[:, vs], po[:])
                        nc.gpsimd.tensor_add(
                            ot[:, vs], ot[:, vs], bias_j_sb[:, vs])
                    if tidx == NT - 1:
                        # final tile: store vh0 as soon as it's ready, on
                        # the ACT ring (free of DMAs by now) so the
                        # kernel-end chain never queues behind the last
                        # 512KB stores draining on the SP ring
                        nc.scalar.dma_start(
                            out_d[tidx * 128:(tidx + 1) * 128, vs], ot[:, vs])
                    elif vh == 1:
                        # full 512KB row-block store once both halves are
                        # evac'd: half the HWDGE issues + sem receipts;
                        # last group drains on the ACT ring
                        eng = nc.scalar if tidx >= NT - 4 else nc.sync
                        eng.dma_start(
                            out_d[tidx * 128:(tidx + 1) * 128, :], ot[:, :])

                ots_all = {}

                def get_ot(tg, pi):
                    key = (tg, pi)
                    if key not in ots_all:
                        ots_all[key] = out_pool.tile([128, V], F32,
                                                     name="ot", tag="ot")
                    return ots_all[key]

                # ---- interleaved startup: tg0's vh0 runs jc-OUTER as 4
                # open PSUM groups, with the jc2/jc3 projections WOVEN
                # BETWEEN prologue rounds. The PE queue is in-order, so
                # emitting all projections first would park the ready
                # prologue round behind proj matmuls stalled on the packB
                # arrival; interleaving keeps PE fed from the first 640KB
                # of loads onward. (4 prologue tiles, not 8: the woven
                # projections need 2 free PSUM banks while pos[] is live,
                # and pos banks only release after the last round.) ----
                if const_slab:
                    for jc in range(JC):
                        proj_jc(jc)
                else:
                    proj_jc(0)
                    proj_jc(1)
                    pos = [ps_mm.tile([128, 512], F32, tag="mm",
                                      name=f"po{i}")
                           for i in range(4)]

                    def prol_round(jc):
                        for pi in range(4):
                            nc.tensor.matmul(
                                pos[pi][:],
                                slab0[:, jc, pi * 128:(pi + 1) * 128],
                                w_joint_sb[:, 0, jc, :],
                                start=(jc == 0), stop=(jc == JC - 1))

                    prol_round(0)
                    proj_jc(2)
                    prol_round(1)
                    proj_jc(3)
                    prol_round(2)
                    prol_round(3)
                    for pi in range(4):
                        evac_store(0, pi, 0, pos[pi], get_ot(0, pi))

                for tg in range(TG):
                    if const_slab:
                        slab = slab0
                    else:
                        if tg + 2 < TG:
                            make_slab(tg + 2)
                        slab = slabs.pop(tg)

                    def half_tile(pi, vh):
                        ot = get_ot(tg, pi)
                        po = ps_mm.tile([128, 512], F32, tag="mm")
                        for jc in range(JC):
                            nc.tensor.matmul(
                                po[:],
                                slab[:, jc, pi * 128:(pi + 1) * 128],
                                w_joint_sb[:, vh, jc, :],
                                start=(jc == 0), stop=(jc == JC - 1))
                        evac_store(tg, pi, vh, po, ot)

                    if tg == 0 and not const_slab:
                        # vh0 already computed in the jc-outer prologue
                        order = [(pi, 1) for pi in range(4)]
                    else:
                        order = [(pi, vh) for pi in range(4) for vh in range(2)]
                    for pi, vh in order:
                        half_tile(pi, vh)

            loop_cm = tc.For_i(0, loop_k) if loop_k else contextlib.nullcontext()
            if hoist_loads and loop_k:
                views = emit_loads()
                with loop_cm:
                    emit_compute(views)
            else:
                with loop_cm:
                    emit_compute(emit_loads())
    nc.compile()
    return nc


def _get_nc():
    if "nc" not in _cache:
        _cache["nc"] = _build()
    return _cache["nc"]


def make_in_maps(enc_out, pred_out, W_enc, b_enc, W_pred, b_pred, W_joint, b_joint):
    enc_f32 = np.asarray(enc_out, dtype=np.float32)
    pred_f32 = np.asarray(pred_out, dtype=np.float32)
    # weights packed [p, jc, ec, 128]; jc=0 chunk goes in packA, rest in packB
    w_enc = np.asarray(W_enc, dtype=np.float32).reshape(EC, 128, JC, 128) \
        .transpose(1, 2, 0, 3).astype(NP_BF16)
    w_pred = np.asarray(W_pred, dtype=np.float32).reshape(EC, 128, JC, 128) \
        .transpose(1, 2, 0, 3).astype(NP_BF16)
    # w_joint packed [p, vh, jc, 512] so per-vh halves are contiguous DMAs
    w_joint = np.ascontiguousarray(
        np.asarray(W_joint, dtype=np.float32).reshape(JC, 128, 2, 512)
        .transpose(1, 2, 0, 3).reshape(128, 2 * JC * 512)).astype(NP_BF16)
    bias_ep = ((np.asarray(b_enc, dtype=np.float32)
                + np.asarray(b_pred, dtype=np.float32))
               .reshape(JC, 128).T).astype(NP_BF16)
    bias_j = np.ascontiguousarray(
        np.broadcast_to(np.asarray(b_joint, dtype=np.float32), (128, V))
    ).astype(NP_BF16)
    packB = np.ascontiguousarray(np.concatenate(
        [w_enc[:, 1:].reshape(128, -1), w_pred[:, 1:].reshape(128, -1)],
        axis=1))
    predT = {}
    for b in range(B):
        predT[b] = pred_f32[b].reshape(U, EC, 128).transpose(2, 1, 0) \
            .reshape(128, EC * U).astype(NP_BF16)
    in_maps = []
    packA_cache = {}
    for c in range(N_CORES):
        b, toff = c // 2, (c % 2) * TL
        key = (b, toff)
        if key not in packA_cache:
            enc_slab = enc_f32[b, toff:toff + TL, :].reshape(TL, EC, 128) \
                .transpose(2, 1, 0).reshape(128, EC * TL).astype(NP_BF16)
            packA_cache[key] = np.ascontiguousarray(np.concatenate(
                [enc_slab, w_enc[:, 0].reshape(128, -1), predT[b],
                 w_pred[:, 0].reshape(128, -1), bias_ep], axis=1))
        in_maps.append({
            "packA": packA_cache[key], "packB": packB,
            "w_joint": w_joint, "bias_j": bias_j,
        })
    return in_maps


def assemble(results):
    # core order is (b0,t0-127), (b0,t128-255), (b1,t0-127), ... so a
    # straight row-stack is already (B, T, U, V)
    return np.concatenate([r["out"] for r in results], axis=0).reshape(B, T, U, V)


def _axon_active():
    try:
        from concourse.bass_utils import axon_active
        return axon_active()
    except Exception:
        return False


def _get_fast_runner(nc):
    """Cached jit dispatch (axon path). Same mechanism as
    bass2jax.run_bass_via_pjrt, built once so repeat kernel() calls skip
    the per-call trace/lower/compile."""
    if "runner" in _cache:
        return _cache["runner"]

    import jax
    from jax.sharding import Mesh, PartitionSpec, NamedSharding
    from jax.experimental.shard_map import shard_map
    from concourse.bass2jax import (
        _bass_exec_p, install_neuronx_cc_hook, partition_id_tensor)

    install_neuronx_cc_hook()
    partition_name = nc.partition_id_tensor.name if nc.partition_id_tensor else None
    in_names, out_names, out_avals, zero_outs = [], [], [], []
    for alloc in nc.m.functions[0].allocations:
        if not isinstance(alloc, mybir.MemoryLocationSet):
            continue
        name = alloc.memorylocations[0].name
        if alloc.kind == "ExternalInput":
            if name != partition_name:
                in_names.append(name)
        elif alloc.kind == "ExternalOutput":
            shape = tuple(alloc.tensor_shape)
            dtype = mybir.dt.np(alloc.dtype)
            out_names.append(name)
            out_avals.append(jax.core.ShapedArray(shape, dtype))
            zero_outs.append(np.zeros(shape, dtype))
    n_params = len(in_names)
    n_outs = len(out_avals)
    all_names = in_names + out_names
    if partition_name is not None:
        all_names = all_names + [partition_name]

    def _body(*args):
        operands = list(args)
        if partition_name is not None:
            operands.append(partition_id_tensor())
        outs = _bass_exec_p.bind(
            *operands, out_avals=tuple(out_avals), in_names=tuple(all_names),
            out_names=tuple(out_names), lowering_input_output_aliases=(),
            sim_require_finite=True, sim_require_nnan=True, nc=nc)
        return tuple(outs)

    devices = jax.devices()[:N_CORES]
    mesh = Mesh(np.asarray(devices), ("core",))
    sharded = jax.jit(
        shard_map(_body, mesh=mesh,
                  in_specs=(PartitionSpec("core"),) * (n_params + n_outs),
                  out_specs=(PartitionSpec("core"),) * n_outs,
                  check_rep=False),
        keep_unused=True)
    sh = NamedSharding(mesh, PartitionSpec("core"))
    zeros_dev = [
        jax.device_put(np.zeros((N_CORES * z.shape[0], *z.shape[1:]), z.dtype), sh)
        for z in zero_outs]

    oi = out_names.index("out")

    def run(in_maps):
        concat_in = [
            jax.device_put(
                np.concatenate([in_maps[c][n] for c in range(N_CORES)], axis=0), sh)
            for n in in_names]
        outs = sharded(*concat_in, *zeros_dev)
        # core order is (b0,t0-127), (b0,t128-255), (b1,t0-127), ... so the
        # row-sharded global array is already (B*T*U, V): one host gather,
        # zero-copy reshape
        return np.asarray(outs[oi]).reshape(B, T, U, V)

    _cache["runner"] = run
    return run


def kernel(enc_out, pred_out, W_enc, b_enc, W_pred, b_pred, W_joint, b_joint):
    nc = _get_nc()
    in_maps = make_in_maps(enc_out, pred_out, W_enc, b_enc, W_pred, b_pred,
                           W_joint, b_joint)
    if _axon_active():
        return _get_fast_runner(nc)(in_maps)
    results = bass_utils.run_bass_kernel_spmd(
        nc, in_maps, list(range(N_CORES))).results
    return assemble(results)


# revision 49
# speedup vs baseline: 1.0184x; 1.0144x over previous
"""RNN-T joint network kernel for 8 Trainium2 NeuronCores.

logits[b,t,u,v] = tanh(enc_out[b,t]@W_enc + b_enc + pred_out[b,u]@W_pred + b_pred) @ W_joint + b_joint

Sharding: (B x T-half) split 8 ways -- core c computes b = c//2,
t in [(c%2)*128, (c%2)*128+128). Each core emits its (128 t, 64 u, 1024 v)
logit slab; stacking the 8 per-core slabs along rows IS the final
(4, 256, 64, 1024) tensor (gather is a reshape, no transpose).

Per-core dataflow (activations kept transposed, J on partitions, bf16
compute with f32 PSUM accumulate and f32 output):
  loads        : need-ordered small DMAs; latency-critical pieces (packA,
                 w_joint vh0) on the ACT HWDGE ring, bulk (packB, bias_j,
                 w_joint vh1 tail) on the gpsimd SWDGE queue; the SP HWDGE
                 ring is reserved for output stores
  encT/predT   : transposed on the HOST in make_in_maps, DMA'd directly
                 in [E-part, row] layout (no on-device transposes)
  enc/pred proj: PE bf16 matmuls; weights host-packed so the jc=0 chunks
                 land first and proj-jc0 starts ~1.5us in
  joint        : DVE broadcast-add (0-step APs) + ACT tanh -> bf16 slab;
                 slab0+slab1 ride per-jc behind the projections, later
                 slabs are produced two groups ahead of their tiles
  prologue     : tg0's vh0 runs jc-OUTER as 4 open PSUM groups with the
                 jc2/jc3 projections WOVEN BETWEEN rounds -- the PE queue
                 is in-order, so un-interleaved emission would park ready
                 prologue rounds behind proj matmuls stalled on the packB
                 arrival; PE starts after only ~640KB of loads
  logits       : PE bf16 matmuls, w_joint streaming as moving operand;
                 PSUM-evac + b_joint (bf16) add on DVE for even tiles,
                 ACT-copy + GPSIMD-add for odd (all-DVE for the last 8)
  store        : full 512KB row-block HWDGE DMA once both vocab halves of
                 a tile are evac'd (half the issue/semaphore cost); the
                 last group's stores drain on the by-then-idle ACT ring
                 (splitting the final ~3.5MB drain across both rings) and
                 the final tile stores in small chunks, so the kernel-end
                 store+receipt chain is as short as possible

_build(loop_k=K) wraps the whole body in a hardware For_i loop executing
the identical kernel K times back-to-back -- used by test.py to measure
steady-state per-iteration HW time with the dispatch overhead cancelled.
"""

import sys

for _p in ("/opt/trn_rl_repo", "/root/.axon_site/_ro/trn_rl_repo"):
    if _p not in sys.path:
        sys.path.insert(0, _p)

import numpy as np

import concourse.tile as tile
from concourse import bacc, mybir
from concourse import bass_utils
from concourse.ap import AP

F32 = mybir.dt.float32
BF16 = mybir.dt.bfloat16
NP_BF16 = mybir.dt.np(BF16)
TANH = mybir.ActivationFunctionType.Tanh
ADD = mybir.AluOpType.add

N_CORES = 8
B, T, U = 4, 256, 64
TL = T // 2                # 128 t's per core (half of one b)
E = P = J = 512
V = 1024
EC = E // 128              # 4 e-chunks (contraction for projections)
JC = J // 128              # 4 j-chunks (contraction for joint matmul)
NROW = TL * U              # 8192 output rows per core
NT = NROW // 128           # 64 output tiles of 128 rows
TG = 16                    # t-groups (8 t's each -> slab of 512 rows)

_cache = {}


def _build(loop_k=None, no_store=False, const_slab=False, hoist_loads=False):
    nc = bacc.Bacc("TRN2", target_bir_lowering=False, debug=False,
                   num_devices=N_CORES)

    # packA: everything the jc=0 projection + first slab chunk needs:
    # [encT (EC*TL) | w_enc jc0 (EC*128) | predT (EC*U) | w_pred jc0
    # (EC*128) | bias_ep (JC, bf16)], loaded as two DMAs
    PA = EC * TL + EC * U + 2 * EC * 128 + JC
    packA_d = nc.dram_tensor("packA", [128, PA], BF16, kind="ExternalInput").ap()
    # packB: w_enc jc1-3 | w_pred jc1-3
    PB = 2 * (JC - 1) * EC * 128
    packB_d = nc.dram_tensor("packB", [128, PB], BF16, kind="ExternalInput").ap()
    w_joint_d = nc.dram_tensor("w_joint", [128, 2 * JC * 512], BF16,
                               kind="ExternalInput").ap()
    bias_j_d = nc.dram_tensor("bias_j", [128, V], BF16, kind="ExternalInput").ap()
    out_d = nc.dram_tensor("out", [NROW, V], F32, kind="ExternalOutput").ap()

    with tile.TileContext(nc) as tc:
        with (
            tc.tile_pool(name="const", bufs=1) as cp,
            tc.tile_pool(name="psum_mm", bufs=8, space="PSUM") as ps_mm,
            tc.tile_pool(name="slab", bufs=4) as slab_pool,
            tc.tile_pool(name="outp", bufs=12) as out_pool,
        ):
            import contextlib

            def emit_loads():
                # ---- constant loads: small, need-ordered DMAs (fixed
                # ~0.6us HWDGE issue cost each) so each pipeline stage's
                # bytes land just-in-time ----
                o0 = EC * TL            # end of encT
                o1 = o0 + EC * 128      # end of w_enc jc0
                o2 = o1 + EC * U        # end of predT
                o3 = o2 + EC * 128      # end of w_pred jc0
                # all loads on the ACT HWDGE ring so they never share a
                # FIFO with the output stores (which own the SP ring)
                packA_sb = cp.tile([128, PA], BF16)
                nc.scalar.dma_start(packA_sb[:, 0:o1], packA_d[:, 0:o1])
                nc.scalar.dma_start(packA_sb[:, o1:PA], packA_d[:, o1:PA])
                w_joint_sb = cp.tile([128, 2, JC, 512], BF16)
                wj_re = w_joint_d.rearrange("p (h c v) -> p h c v", h=2, c=JC)
                nc.scalar.dma_start(w_joint_sb[:, 0, 0], wj_re[:, 0, 0])
                packB_sb = cp.tile([128, PB], BF16)
                nc.gpsimd.dma_start(packB_sb[:, 0:PB // 2], packB_d[:, 0:PB // 2])
                nc.gpsimd.dma_start(packB_sb[:, PB // 2:PB], packB_d[:, PB // 2:PB])
                nc.scalar.dma_start(w_joint_sb[:, 0, 1:4], wj_re[:, 0, 1:4])
                bias_j_sb = cp.tile([128, V], BF16)
                nc.gpsimd.dma_start(bias_j_sb[:], bias_j_d[:])
                nc.scalar.dma_start(w_joint_sb[:, 1, 0], wj_re[:, 1, 0])
                nc.gpsimd.dma_start(w_joint_sb[:, 1, 1:4], wj_re[:, 1, 1:4])

                # views into the packs
                encT = packA_sb[:, 0:o0].rearrange("p (e t) -> p e t", e=EC)
                we0 = packA_sb[:, o0:o1].rearrange("p (e k) -> p e k", e=EC)
                predT = packA_sb[:, o1:o2].rearrange("p (e u) -> p e u", e=EC)
                wp0 = packA_sb[:, o2:o3].rearrange("p (e k) -> p e k", e=EC)
                bias_ep_sb = packA_sb[:, o3:o3 + JC]
                web = packB_sb[:, 0:PB // 2].rearrange(
                    "p (c e k) -> p c e k", c=JC - 1, e=EC)
                wpb = packB_sb[:, PB // 2:PB].rearrange(
                    "p (c e k) -> p c e k", c=JC - 1, e=EC)

                def w_enc_ap(jc, ec):
                    return we0[:, ec, :] if jc == 0 else web[:, jc - 1, ec, :]

                def w_pred_ap(jc, ec):
                    return wp0[:, ec, :] if jc == 0 else wpb[:, jc - 1, ec, :]

                return (encT, predT, w_enc_ap, w_pred_ap, bias_ep_sb,
                        w_joint_sb, bias_j_sb)

            def emit_compute(views):
                (encT, predT, w_enc_ap, w_pred_ap, bias_ep_sb,
                 w_joint_sb, bias_j_sb) = views
                # ---- projections (transposed layout [j, row]), interleaved
                # per-jc so the first slab chunk is ready ASAP ----
                encP = cp.tile([128, JC, TL], BF16)
                predP = cp.tile([128, JC, U], BF16)
                slab0 = slab_pool.tile([128, JC, 512], BF16)
                slab1 = slab_pool.tile([128, JC, 512], BF16)

                def slab_add_tanh(slab, tg, jc):
                    t0 = tg * 8
                    # [128, 8t, 64u] = pred[., u] (bcast t) + enc[., t] (bcast u)
                    p_ap = predP[:, jc, :]
                    in0 = AP(p_ap.tensor, p_ap.offset,
                             [p_ap.ap[0], [0, 8], [1, U]])
                    e_ap = encP[:, jc, t0:t0 + 8]
                    in1 = AP(e_ap.tensor, e_ap.offset,
                             [e_ap.ap[0], [1, 8], [0, U]])
                    dst = slab[:, jc, :].rearrange("p (t u) -> p t u", t=8)
                    nc.vector.tensor_tensor(dst, in0, in1, ADD)
                    nc.scalar.activation(slab[:, jc, :], slab[:, jc, :], TANH)

                def proj_jc(jc):
                    pe = ps_mm.tile([128, 512], F32, tag="mm", name="pe")
                    for ec in range(EC):
                        nc.tensor.matmul(pe[:, 0:TL],
                                         w_enc_ap(jc, ec),
                                         encT[:, ec, :],
                                         start=(ec == 0), stop=(ec == EC - 1))
                    nc.scalar.copy(encP[:, jc, :], pe[:, 0:TL])
                    pp = ps_mm.tile([128, 512], F32, tag="mm", name="pp")
                    for ec in range(EC):
                        nc.tensor.matmul(pp[:, 0:U],
                                         w_pred_ap(jc, ec),
                                         predT[:, ec, :],
                                         start=(ec == 0), stop=(ec == EC - 1))
                    # fused (b_enc + b_pred) bias add during PSUM evacuation
                    # (bias broadcast along u via a 0-step AP)
                    b_ap = bias_ep_sb[:, jc:jc + 1]
                    b_bc = AP(b_ap.tensor, b_ap.offset, [b_ap.ap[0], [0, U]])
                    nc.vector.tensor_tensor(predP[:, jc, :], pp[:, 0:U],
                                            b_bc, ADD)
                    # first two slab chunks ride right behind their
                    # projections (both consumed by the jc-outer prologue)
                    slab_add_tanh(slab0, 0, jc)
                    slab_add_tanh(slab1, 1, jc)

                # ---- main loop: per t-group of 8 -> slab -> 4 out tiles.
                # Slabs are produced one group ahead of their tiles so
                # DVE/ACT slab work never gates the PE matmul stream. ----
                slabs = {0: slab0, 1: slab1}

                def make_slab(tg):
                    s = slab_pool.tile([128, JC, 512], BF16, name="slab",
                                       tag="slab")
                    for jc in range(JC):
                        slab_add_tanh(s, tg, jc)
                    slabs[tg] = s

                def evac_store(tg, pi, vh, po, ot):
                    tidx = tg * 4 + pi
                    vs = slice(vh * 512, (vh + 1) * 512)
                    if no_store:
                        # timing ablation: tiny evac+store keeps the full
                        # dependency chain alive but removes ~all DVE/DMA load
                        vs16 = slice(vh * 512, vh * 512 + 16)
                        nc.vector.tensor_tensor(
                            ot[:, vs16], po[:, 0:16], bias_j_sb[:, vs16], ADD)
                        nc.sync.dma_start(
                            out_d[tidx * 128:(tidx + 1) * 128, vs16],
                            ot[:, vs16])
                        return
                    if tidx == NT - 1 and vh == 1:
                        # final half-tile: evac+store in two 256-col chunks
                        # so the kernel-end store chain is as short as
                        # possible (smaller final transfer + receipt)
                        for h in range(2):
                            hs = slice(vh * 512 + h * 256,
                                       vh * 512 + (h + 1) * 256)
                            ps = slice(h * 256, (h + 1) * 256)
                            nc.vector.tensor_tensor(
                                ot[:, hs], po[:, ps], bias_j_sb[:, hs], ADD)
                            nc.scalar.dma_start(
                                out_d[tidx * 128:(tidx + 1) * 128, hs],
                                ot[:, hs])
                        return
                    if tidx % 2 == 0 or tidx >= NT - 8:
                        # DVE: fused PSUM evac + b_joint add
                        nc.vector.tensor_tensor(
                            ot[:, vs], po[:], bias_j_sb[:, vs], ADD)
                    else:
                        # ACT evacuates, GPSIMD adds bias in place
                        nc.scalar.copy(ot[:, vs], po[:])
                        nc.gpsimd.tensor_add(
                            ot[:, vs], ot[:, vs], bias_j_sb[:, vs])
                    if tidx == NT - 1:
                        # final tile: store vh0 as soon as it's ready, on
                        # the ACT ring (free of DMAs by now) so the
                        # kernel-end chain never queues behind the last
                        # 512KB stores draining on the SP ring
                        nc.scalar.dma_start(
                            out_d[tidx * 128:(tidx + 1) * 128, vs], ot[:, vs])
                    elif vh == 1:
                        # full 512KB row-block store once both halves are
                        # evac'd: half the HWDGE issues + sem receipts;
                        # last group drains on the ACT ring
                        eng = nc.scalar if tidx >= NT - 4 else nc.sync
                        eng.dma_start(
                            out_d[tidx * 128:(tidx + 1) * 128, :], ot[:, :])

                ots_all = {}

                def get_ot(tg, pi):
                    key = (tg, pi)
                    if key not in ots_all:
                        ots_all[key] = out_pool.tile([128, V], F32,
                                                     name="ot", tag="ot")
                    return ots_all[key]

                # ---- interleaved startup: tg0's vh0 runs jc-OUTER as 4
                # open PSUM groups, with the jc2/jc3 projections WOVEN
                # BETWEEN prologue rounds. The PE queue is in-order, so
                # emitting all projections first would park the ready
                # prologue round behind proj matmuls stalled on the packB
                # arrival; interleaving keeps PE fed from the first 640KB
                # of loads onward. (4 prologue tiles, not 8: the woven
                # projections need 2 free PSUM banks while pos[] is live,
                # and pos banks only release after the last round.) ----
                if const_slab:
                    for jc in range(JC):
                        proj_jc(jc)
                else:
                    proj_jc(0)
                    proj_jc(1)
                    pos = [ps_mm.tile([128, 512], F32, tag="mm",
                                      name=f"po{i}")
                           for i in range(4)]

                    def prol_round(jc):
                        for pi in range(4):
                            nc.tensor.matmul(
                                pos[pi][:],
                                slab0[:, jc, pi * 128:(pi + 1) * 128],
                                w_joint_sb[:, 0, jc, :],
                                start=(jc == 0), stop=(jc == JC - 1))

                    prol_round(0)
                    proj_jc(2)
                    prol_round(1)
                    proj_jc(3)
                    prol_round(2)
                    prol_round(3)
                    for pi in range(4):
                        evac_store(0, pi, 0, pos[pi], get_ot(0, pi))

                for tg in range(TG):
                    if const_slab:
                        slab = slab0
                    else:
                        if tg + 2 < TG:
                            make_slab(tg + 2)
                        slab = slabs.pop(tg)

                    def half_tile(pi, vh):
                        ot = get_ot(tg, pi)
                        po = ps_mm.tile([128, 512], F32, tag="mm")
                        for jc in range(JC):
                            nc.tensor.matmul(
                                po[:],
                                slab[:, jc, pi * 128:(pi + 1) * 128],
                                w_joint_sb[:, vh, jc, :],
                                start=(jc == 0), stop=(jc == JC - 1))
                        evac_store(tg, pi, vh, po, ot)

                    if tg == 0 and not const_slab:
                        # vh0 already computed in the jc-outer prologue
                        order = [(pi, 1) for pi in range(4)]
                    else:
                        order = [(pi, vh) for pi in range(4) for vh in range(2)]
                    for pi, vh in order:
                        half_tile(pi, vh)

            loop_cm = tc.For_i(0, loop_k) if loop_k else contextlib.nullcontext()
            if hoist_loads and loop_k:
                views = emit_loads()
                with loop_cm:
                    emit_compute(views)
            else:
                with loop_cm:
                    emit_compute(emit_loads())
    nc.compile()
    return nc


def _get_nc():
    if "nc" not in _cache:
        _cache["nc"] = _build()
    return _cache["nc"]


def make_in_maps(enc_out, pred_out, W_enc, b_enc, W_pred, b_pred, W_joint, b_joint):
    enc_f32 = np.asarray(enc_out, dtype=np.float32)
    pred_f32 = np.asarray(pred_out, dtype=np.float32)
    # weights packed [p, jc, ec, 128]; jc=0 chunk goes in packA, rest in packB
    w_enc = np.asarray(W_enc, dtype=np.float32).reshape(EC, 128, JC, 128) \
        .transpose(1, 2, 0, 3).astype(NP_BF16)
    w_pred = np.asarray(W_pred, dtype=np.float32).reshape(EC, 128, JC, 128) \
        .transpose(1, 2, 0, 3).astype(NP_BF16)
    # w_joint packed [p, vh, jc, 512] so per-vh halves are contiguous DMAs
    w_joint = np.ascontiguousarray(
        np.asarray(W_joint, dtype=np.float32).reshape(JC, 128, 2, 512)
        .transpose(1, 2, 0, 3).reshape(128, 2 * JC * 512)).astype(NP_BF16)
    bias_ep = ((np.asarray(b_enc, dtype=np.float32)
                + np.asarray(b_pred, dtype=np.float32))
               .reshape(JC, 128).T).astype(NP_BF16)
    bias_j = np.ascontiguousarray(
        np.broadcast_to(np.asarray(b_joint, dtype=np.float32), (128, V))
    ).astype(NP_BF16)
    packB = np.ascontiguousarray(np.concatenate(
        [w_enc[:, 1:].reshape(128, -1), w_pred[:, 1:].reshape(128, -1)],
        axis=1))
    predT = {}
    for b in range(B):
        predT[b] = pred_f32[b].reshape(U, EC, 128).transpose(2, 1, 0) \
            .reshape(128, EC * U).astype(NP_BF16)
    in_maps = []
    packA_cache = {}
    for c in range(N_CORES):
        b, toff = c // 2, (c % 2) * TL
        key = (b, toff)
        if key not in packA_cache:
            enc_slab = enc_f32[b, toff:toff + TL, :].reshape(TL, EC, 128) \
                .transpose(2, 1, 0).reshape(128, EC * TL).astype(NP_BF16)
            packA_cache[key] = np.ascontiguousarray(np.concatenate(
                [enc_slab, w_enc[:, 0].reshape(128, -1), predT[b],
                 w_pred[:, 0].reshape(128, -1), bias_ep], axis=1))
        in_maps.append({
            "packA": packA_cache[key], "packB": packB,
            "w_joint": w_joint, "bias_j": bias_j,
        })
    return in_maps


def assemble(results):
    # core order is (b0,t0-127), (b0,t128-255), (b1,t0-127), ... so a
    # straight row-stack is already (B, T, U, V)
    return np.concatenate([r["out"] for r in results], axis=0).reshape(B, T, U, V)


def _axon_active():
    try:
        from concourse.bass_utils import axon_active
        return axon_active()
    except Exception:
        return False


def _get_fast_runner(nc):
    """Cached jit dispatch (axon path). Same mechanism as
    bass2jax.run_bass_via_pjrt, built once so repeat kernel() calls skip
    the per-call trace/lower/compile."""
    if "runner" in _cache:
        return _cache["runner"]

    import jax
    from jax.sharding import Mesh, PartitionSpec, NamedSharding
    from jax.experimental.shard_map import shard_map
    from concourse.bass2jax import (
        _bass_exec_p, install_neuronx_cc_hook, partition_id_tensor)

    install_neuronx_cc_hook()
    partition_name = nc.partition_id_tensor.name if nc.partition_id_tensor else None
    in_names, out_names, out_avals, zero_outs = [], [], [], []
    for alloc in nc.m.functions[0].allocations:
        if not isinstance(alloc, mybir.MemoryLocationSet):
            continue
        name = alloc.memorylocations[0].name
        if alloc.kind == "ExternalInput":
            if name != partition_name:
                in_names.append(name)
        elif alloc.kind == "ExternalOutput":
            shape = tuple(alloc.tensor_shape)
            dtype = mybir.dt.np(alloc.dtype)
            out_names.append(name)
            out_avals.append(jax.core.ShapedArray(shape, dtype))
            zero_outs.append(np.zeros(shape, dtype))
    n_params = len(in_names)
    n_outs = len(out_avals)
    all_names = in_names + out_names
    if partition_name is not None:
        all_names = all_names + [partition_name]

    def _body(*args):
        operands = list(args)
        if partition_name is not None:
            operands.append(partition_id_tensor())
        outs = _bass_exec_p.bind(
            *operands, out_avals=tuple(out_avals), in_names=tuple(all_names),
            out_names=tuple(out_names), lowering_input_output_aliases=(),
            sim_require_finite=True, sim_require_nnan=True, nc=nc)
        return tuple(outs)

    devices = jax.devices()[:N_CORES]
    mesh = Mesh(np.asarray(devices), ("core",))
    sharded = jax.jit(
        shard_map(_body, mesh=mesh,
                  in_specs=(PartitionSpec("core"),) * (n_params + n_outs),
                  out_specs=(PartitionSpec("core"),) * n_outs,
                  check_rep=False),
        keep_unused=True)
    sh = NamedSharding(mesh, PartitionSpec("core"))
    zeros_dev = [
        jax.device_put(np.zeros((N_CORES * z.shape[0], *z.shape[1:]), z.dtype), sh)
        for z in zero_outs]

    oi = out_names.index("out")

    def run(in_maps):
        concat_in = [
            jax.device_put(
                np.concatenate([in_maps[c][n] for c in range(N_CORES)], axis=0), sh)
            for n in in_names]
        outs = sharded(*concat_in, *zeros_dev)
        # core order is (b0,t0-127), (b0,t128-255), (b1,t0-127), ... so the
        # row-sharded global array is already (B*T*U, V): one host gather,
        # zero-copy reshape
        return np.asarray(outs[oi]).reshape(B, T, U, V)

    _cache["runner"] = run
    return run


def kernel(enc_out, pred_out, W_enc, b_enc, W_pred, b_pred, W_joint, b_joint):
    nc = _get_nc()
    in_maps = make_in_maps(enc_out, pred_out, W_enc, b_enc, W_pred, b_pred,
                           W_joint, b_joint)
    if _axon_active():
        return _get_fast_runner(nc)(in_maps)
    results = bass_utils.run_bass_kernel_spmd(
        nc, in_maps, list(range(N_CORES))).results
    return assemble(results)
